# revision 3
# baseline (speedup 1.0000x reference)
"""Trainium2 Bass kernel for nn_BartPooler_53815940219079 (segment_reduce).

Computes, for each of B*T segments of a [B, S, H] hidden-state tensor:
  feat = concat([segment_max, segment_mean])  -> tanh(feat @ W.T + b)

Strategy (8 NeuronCores, SPMD — one program, per-core data):
  * Host compacts each segment's used tokens into a per-core token stream,
    padding every segment with duplicates of its first token so that each
    segment occupies a whole number of G-token "groups" (plus a compensation
    group whose negative membership weight cancels the duplicate tokens in
    the sum).  Segments are dealt snake-wise across cores by size so all
    cores share one static layout (slot j has the same group range on every
    core).
  * Device, per 128-group tile: grouped max/sum partials on VectorE;
    per-segment means via membership matmuls on TensorE (weights fold in
    1/count); PE transposes of the max partials; per-segment max reduce on
    VectorE; then a fused [2H] x [2H, D] GEMM with bias + tanh.
"""

import numpy as np

import concourse.bacc as bacc
import concourse.mybir as mybir
import concourse.tile as tile
from concourse.bass_utils import run_bass_kernel_spmd
from concourse.masks import make_identity

NCORES = 8
G = 4          # tokens per group
PTILE = 128 * G  # tokens per main tile

B, S, H, T = 16, 4096, 1024, 16
D_OUT = 1024
HB = H // 128  # h-blocks per hidden vector

F32 = mybir.dt.float32


def _build_schedule(parts, turns):
    """Host-side: segment list -> per-core compacted layout (uniform shapes)."""
    Bn, Tn = parts.shape
    segs = []  # (global_row, example, start_token, count)
    for b in range(Bn):
        cum = 0
        for j in range(Tn):
            c = int(parts[b, j])
            if j < int(turns[b]):
                segs.append((b * Tn + j, b, 1 + cum, c))
            cum += c

    # Deal segments to cores snake-wise by descending size: slot j has a
    # similar size on every core, minimizing uniform-padding waste.
    order = sorted(range(len(segs)), key=lambda i: -segs[i][3])
    core_slots = [[] for _ in range(NCORES)]
    for rank, i in enumerate(order):
        rnd, pos = divmod(rank, NCORES)
        c = pos if rnd % 2 == 0 else NCORES - 1 - pos
        core_slots[c].append(segs[i])
    seg_cap = max(len(s) for s in core_slots)

    def groups_needed(cnt):
        g = (cnt + G - 1) // G
        if cnt % G:
            g += 1  # at least one pure-duplicate group for the compensation
        return g

    # Uniform per-slot group counts across cores.
    L = []
    for j in range(seg_cap):
        m = 1
        for c in range(NCORES):
            if j < len(core_slots[c]):
                m = max(m, groups_needed(core_slots[c][j][3]))
        L.append(m)
    A = np.concatenate([[0], np.cumsum(L)]).astype(np.int64)  # slot -> group start
    ngroups = int(A[-1])
    ntiles = (ngroups + 127) // 128
    ngroups_pad = ntiles * 128
    ntok = ngroups_pad * G

    # Per-core token-gather indices (into flat [B*S]) and membership weights.
    tok_idx = np.full((NCORES, ntok), -1, dtype=np.int64)
    member = np.zeros((NCORES, 128, ntiles, seg_cap), dtype=np.float32)
    out_map = np.full((NCORES, seg_cap), -1, dtype=np.int64)
    for c in range(NCORES):
        for j, (grow, b, s0, cnt) in enumerate(core_slots[c]):
            out_map[c, j] = grow
            g0 = int(A[j])
            nfull, rem = divmod(cnt, G)
            base = b * S + s0
            t0 = base  # first token, used as the harmless duplicate
            pos = g0 * G
            tok_idx[c, pos:pos + cnt] = np.arange(base, base + cnt)
            pos += cnt
            npure = L[j] - nfull - (1 if rem else 0)
            r = (G - rem) % G
            if r:
                tok_idx[c, pos:pos + r] = t0
                pos += r
            if npure:
                tok_idx[c, pos:pos + npure * G] = t0
            # weights: real groups 1/cnt, pure groups -r/(npure*G*cnt)
            inv = 1.0 / cnt
            nreal = nfull + (1 if rem else 0)
            for k in range(nreal):
                g = g0 + k
                member[c, g % 128, g // 128, j] = inv
            beta = -r / (npure * G) * inv if (npure and r) else 0.0
            for k in range(npure):
                g = g0 + nreal + k
                member[c, g % 128, g // 128, j] = beta
    return {
        "core_slots": core_slots,
        "seg_cap": seg_cap,
        "L": L,
        "A": A,
        "ntiles": ntiles,
        "ntok": ntok,
        "tok_idx": tok_idx,
        "member": member,
        "out_map": out_map,
        "nrows": Bn * Tn,
    }


def _build_program(ntiles, seg_cap, A, L):
    """Emit the SPMD Bass program (identical for all cores)."""
    ntok = ntiles * PTILE
    ngp = ntiles * 128  # padded group count

    nc = bacc.Bacc("TRN2", target_bir_lowering=False, debug=False,
                   num_devices=NCORES)
    hid = nc.dram_tensor("hid", [ntok, H], F32, kind="ExternalInput")
    mem = nc.dram_tensor("mem", [128, ntiles, seg_cap], F32, kind="ExternalInput")
    wt = nc.dram_tensor("wt", [2 * H, D_OUT], F32, kind="ExternalInput")
    brep = nc.dram_tensor("brep", [seg_cap, D_OUT], F32, kind="ExternalInput")
    out = nc.dram_tensor("out", [seg_cap, D_OUT], F32, kind="ExternalOutput")

    with tile.TileContext(nc) as tc:
        with (
            tc.tile_pool(name="const", bufs=1) as constp,
            tc.tile_pool(name="hidp", bufs=3) as hidp,
            tc.tile_pool(name="partial", bufs=3) as partp,
            tc.tile_pool(name="psum_tr", bufs=2, space="PSUM") as trpp,
            tc.tile_pool(name="psum_acc", bufs=1, space="PSUM") as accp,
            tc.tile_pool(name="small", bufs=1) as smallp,
        ):
            ident = constp.tile([128, 128], F32)
            make_identity(nc, ident[:])

            wt_sb = constp.tile([128, 2 * HB, D_OUT], F32)
            nc.sync.dma_start(
                out=wt_sb[:],
                in_=wt[:].rearrange("(kb p) n -> p kb n", p=128),
            )
            brep_sb = constp.tile([seg_cap, D_OUT], F32)
            nc.sync.dma_start(out=brep_sb[:], in_=brep[:])
            mem_sb = constp.tile([128, ntiles, seg_cap], F32)
            nc.sync.dma_start(out=mem_sb[:], in_=mem[:])

            trmax = constp.tile([128, HB, ngp], F32)
            mean_ps = accp.tile([seg_cap, D_OUT], F32, tag="acc")

            for t in range(ntiles):
                ht = hidp.tile([128, G * H], F32)
                nc.sync.dma_start(
                    out=ht[:],
                    in_=hid[t * PTILE:(t + 1) * PTILE, :]
                        .rearrange("(p g) h -> p (g h)", g=G),
                )
                gview = ht[:].rearrange("p (g h) -> p h g", h=H)
                gmax = partp.tile([128, H], F32, tag="gmax")
                gsum = partp.tile([128, H], F32, tag="gsum")
                nc.vector.reduce_max(out=gmax[:], in_=gview,
                                     axis=mybir.AxisListType.X)
                nc.vector.reduce_sum(out=gsum[:], in_=gview,
                                     axis=mybir.AxisListType.X)
                # segment means accumulate on PE (weights already carry 1/cnt)
                for nh in range(2):
                    nc.tensor.matmul(
                        mean_ps[:, nh * 512:(nh + 1) * 512],
                        lhsT=mem_sb[:, t, :],
                        rhs=gsum[:, nh * 512:(nh + 1) * 512],
                        start=(t == 0),
                        stop=(t == ntiles - 1),
                    )
                # transpose the max partials: [group, h] -> [h, group]
                trp = trpp.tile([128, H], F32, tag="trp")
                for hb in range(HB):
                    nc.tensor.transpose(
                        trp[:, hb * 128:(hb + 1) * 128],
                        gmax[:, hb * 128:(hb + 1) * 128],
                        ident[:],
                    )
                nc.scalar.copy(
                    out=trmax[:, :, t * 128:(t + 1) * 128],
                    in_=trp[:].rearrange("p (b g) -> p b g", g=128),
                )

            # means: PSUM -> SBUF, then transpose to [h, slot] layout
            means = smallp.tile([seg_cap, D_OUT], F32)
            nc.scalar.copy(out=means[:], in_=mean_ps[:])
            tr2 = trpp.tile([128, HB * seg_cap], F32, tag="tr2")
            for hb in range(HB):
                nc.tensor.transpose(
                    tr2[:, hb * seg_cap:(hb + 1) * seg_cap],
                    means[:, hb * 128:(hb + 1) * 128],
                    ident[:seg_cap, :seg_cap],
                )
            meansT = smallp.tile([128, HB, seg_cap], F32)
            nc.scalar.copy(
                out=meansT[:],
                in_=tr2[:].rearrange("p (b j) -> p b j", j=seg_cap),
            )

            # per-segment max over its group range (all h-blocks at once)
            maxT = smallp.tile([128, seg_cap, HB], F32)
            for j in range(seg_cap):
                a, l = int(A[j]), int(L[j])
                nc.vector.reduce_max(
                    out=maxT[:, j, :],
                    in_=trmax[:, :, a:a + l],
                    axis=mybir.AxisListType.X,
                )

            # GEMM: out[slot, n] = sum_k featT[k, slot] * wt[k, n]
            out_ps = accp.tile([seg_cap, D_OUT], F32, tag="acc")
            for nh in range(2):
                nsl = slice(nh * 512, (nh + 1) * 512)
                for kb in range(2 * HB):
                    lhsT = maxT[:, :, kb] if kb < HB else meansT[:, kb - HB, :]
                    nc.tensor.matmul(
                        out_ps[:, nsl],
                        lhsT=lhsT,
                        rhs=wt_sb[:, kb, nsl],
                        start=(kb == 0),
                        stop=(kb == 2 * HB - 1),
                    )

            osb = smallp.tile([seg_cap, D_OUT], F32)
            nc.vector.tensor_add(out=osb[:], in0=out_ps[:], in1=brep_sb[:])
            osb2 = smallp.tile([seg_cap, D_OUT], F32)
            nc.scalar.activation(osb2[:], osb[:],
                                 mybir.ActivationFunctionType.Tanh)
            nc.sync.dma_start(out=out[:], in_=osb2[:])

    nc.compile()
    return nc


def _build_in_maps(sched, hidden_states, W, b):
    seg_cap, ntiles = sched["seg_cap"], sched["ntiles"]
    flat = np.ascontiguousarray(
        np.asarray(hidden_states, dtype=np.float32)).reshape(B * S, H)
    wt_np = np.ascontiguousarray(np.asarray(W, dtype=np.float32).T)  # [2H, D]
    brep_np = np.ascontiguousarray(
        np.broadcast_to(np.asarray(b, dtype=np.float32), (seg_cap, D_OUT)))

    in_maps = []
    for c in range(NCORES):
        idx = sched["tok_idx"][c]
        stream = np.zeros((sched["ntok"], H), dtype=np.float32)
        valid = idx >= 0
        stream[valid] = flat[idx[valid]]
        memc = np.ascontiguousarray(
            sched["member"][c].reshape(128, ntiles, seg_cap))
        in_maps.append({
            "hid": stream,
            "mem": memc,
            "wt": wt_np,
            "brep": brep_np,
        })
    return in_maps


def kernel(hidden_states, W, b, turns, parts):
    parts = np.asarray(parts)
    turns = np.asarray(turns)

    sched = _build_schedule(parts, turns)
    nc = _build_program(sched["ntiles"], sched["seg_cap"],
                        sched["A"], sched["L"])
    in_maps = _build_in_maps(sched, hidden_states, W, b)

    res = run_bass_kernel_spmd(nc, in_maps, list(range(NCORES)))

    full = np.zeros((sched["nrows"], D_OUT), dtype=np.float32)
    for c in range(NCORES):
        oc = res.results[c]["out"]
        for j in range(sched["seg_cap"]):
            g = sched["out_map"][c, j]
            if g >= 0:
                full[g] = oc[j]
    return full


# revision 6
# speedup vs baseline: 1.7017x; 1.7017x over previous
"""Trainium2 Bass kernel for nn_BartPooler_53815940219079 (segment_reduce).

Computes, for each of B*T segments of a [B, S, H] hidden-state tensor:
  feat = concat([segment_max, segment_mean])  -> tanh(feat @ W.T + b)

Strategy (8 NeuronCores, SPMD — one program, per-core data):
  * Host compacts each segment's used tokens into a per-core token stream,
    padding every segment with duplicates of its first token so that each
    segment occupies a whole number of G-token "groups" (plus a compensation
    group whose negative membership weight cancels the duplicate tokens in
    the sum).  Segments are dealt snake-wise across cores by size so all
    cores share one static layout (slot j has the same group range on every
    core).
  * Device, per 128-group tile: grouped max/sum partials on VectorE;
    per-segment means via membership matmuls on TensorE (weights fold in
    1/count); PE transposes of the max partials; per-segment max reduce on
    VectorE; then a fused [2H] x [2H, D] GEMM with bias + tanh.
"""

import numpy as np

import concourse.bacc as bacc
import concourse.mybir as mybir
import concourse.tile as tile
from concourse.bass_utils import run_bass_kernel_spmd
from concourse.masks import make_identity

NCORES = 8
G = 4          # tokens per group
PTILE = 128 * G  # tokens per main tile

B, S, H, T = 16, 4096, 1024, 16
D_OUT = 1024
HB = H // 128  # h-blocks per hidden vector

F32 = mybir.dt.float32


def _build_schedule(parts, turns):
    """Host-side: segment list -> per-core compacted layout (uniform shapes)."""
    Bn, Tn = parts.shape
    segs = []  # (global_row, example, start_token, count)
    for b in range(Bn):
        cum = 0
        for j in range(Tn):
            c = int(parts[b, j])
            if j < int(turns[b]):
                segs.append((b * Tn + j, b, 1 + cum, c))
            cum += c

    # Deal segments to cores by size rank: slot j holds the 8 segments of
    # ranks [8j, 8j+8), one per core, so the uniform per-slot group count
    # L[j] (max over cores) is as tight as possible.
    order = sorted(range(len(segs)), key=lambda i: -segs[i][3])
    core_slots = [[] for _ in range(NCORES)]
    for rank, i in enumerate(order):
        core_slots[rank % NCORES].append(segs[i])
    seg_cap = max(len(s) for s in core_slots)

    def groups_needed(cnt):
        g = (cnt + G - 1) // G
        if cnt % G:
            g += 1  # at least one pure-duplicate group for the compensation
        return g

    # Uniform per-slot group counts across cores.
    L = []
    for j in range(seg_cap):
        m = 1
        for c in range(NCORES):
            if j < len(core_slots[c]):
                m = max(m, groups_needed(core_slots[c][j][3]))
        L.append(m)
    A = np.concatenate([[0], np.cumsum(L)]).astype(np.int64)  # slot -> group start
    ngroups = int(A[-1])
    ntiles = (ngroups + 127) // 128
    ngroups_pad = ntiles * 128
    ntok = ngroups_pad * G

    # Per-core token-gather indices (into flat [B*S]) and membership weights.
    tok_idx = np.full((NCORES, ntok), -1, dtype=np.int64)
    member = np.zeros((NCORES, 128, ntiles, seg_cap), dtype=np.float32)
    out_map = np.full((NCORES, seg_cap), -1, dtype=np.int64)
    for c in range(NCORES):
        for j, (grow, b, s0, cnt) in enumerate(core_slots[c]):
            out_map[c, j] = grow
            g0 = int(A[j])
            nfull, rem = divmod(cnt, G)
            base = b * S + s0
            t0 = base  # first token, used as the harmless duplicate
            pos = g0 * G
            tok_idx[c, pos:pos + cnt] = np.arange(base, base + cnt)
            pos += cnt
            npure = L[j] - nfull - (1 if rem else 0)
            r = (G - rem) % G
            if r:
                tok_idx[c, pos:pos + r] = t0
                pos += r
            if npure:
                tok_idx[c, pos:pos + npure * G] = t0
            # weights: real groups 1/cnt, pure groups -r/(npure*G*cnt)
            inv = 1.0 / cnt
            nreal = nfull + (1 if rem else 0)
            for k in range(nreal):
                g = g0 + k
                member[c, g % 128, g // 128, j] = inv
            beta = -r / (npure * G) * inv if (npure and r) else 0.0
            for k in range(npure):
                g = g0 + nreal + k
                member[c, g % 128, g // 128, j] = beta
    return {
        "core_slots": core_slots,
        "seg_cap": seg_cap,
        "L": L,
        "A": A,
        "ntiles": ntiles,
        "ntok": ntok,
        "tok_idx": tok_idx,
        "member": member,
        "out_map": out_map,
        "nrows": Bn * Tn,
    }


def _build_program(ntiles, seg_cap, A, L):
    """Emit the SPMD Bass program (identical for all cores)."""
    ntok = ntiles * PTILE
    ngp = ntiles * 128  # padded group count

    nc = bacc.Bacc("TRN2", target_bir_lowering=False, debug=False,
                   num_devices=NCORES)
    hid = nc.dram_tensor("hid", [ntok, H], F32, kind="ExternalInput")
    mem = nc.dram_tensor("mem", [128, ntiles, seg_cap], F32, kind="ExternalInput")
    wt = nc.dram_tensor("wt", [2 * H, D_OUT], F32, kind="ExternalInput")
    brep = nc.dram_tensor("brep", [seg_cap, D_OUT], F32, kind="ExternalInput")
    out = nc.dram_tensor("out", [seg_cap, D_OUT], F32, kind="ExternalOutput")

    with tile.TileContext(nc) as tc:
        with (
            tc.tile_pool(name="const", bufs=1) as constp,
            tc.tile_pool(name="hidp", bufs=2) as hidp,
            tc.tile_pool(name="partial", bufs=2) as partp,
            tc.tile_pool(name="psum_tr", bufs=2, space="PSUM") as trpp,
            tc.tile_pool(name="psum_acc", bufs=1, space="PSUM") as accp,
            tc.tile_pool(name="small", bufs=1) as smallp,
        ):
            ident = constp.tile([128, 128], F32)
            make_identity(nc, ident[:])

            wt_sb = constp.tile([128, 2 * HB, D_OUT], F32)
            nc.sync.dma_start(
                out=wt_sb[:],
                in_=wt[:].rearrange("(kb p) n -> p kb n", p=128),
            )
            brep_sb = constp.tile([seg_cap, D_OUT], F32)
            nc.sync.dma_start(out=brep_sb[:], in_=brep[:])
            mem_sb = constp.tile([128, ntiles, seg_cap], F32)
            nc.sync.dma_start(out=mem_sb[:], in_=mem[:])

            trmax = constp.tile([128, HB, ngp], F32)
            mean_ps = accp.tile([seg_cap, D_OUT], F32, tag="acc")

            for t in range(ntiles):
                ht = hidp.tile([128, G * H], F32)
                nc.sync.dma_start(
                    out=ht[:],
                    in_=hid[t * PTILE:(t + 1) * PTILE, :]
                        .rearrange("(p g) h -> p (g h)", g=G),
                )
                # Grouped max/sum over G=4 tokens per partition via two
                # contiguous tensor-tensor tree steps (a strided
                # tensor_reduce over the token axis measures ~2x slower).
                gmax = partp.tile([128, H], F32, tag="gmax")
                gsum = partp.tile([128, H], F32, tag="gsum")
                tmx = partp.tile([128, 2 * H], F32, tag="tmx")
                tsm = partp.tile([128, 2 * H], F32, tag="tsm")
                half = G // 2 * H
                nc.vector.tensor_tensor(out=tmx[:], in0=ht[:, :half],
                                        in1=ht[:, half:], op=mybir.AluOpType.max)
                nc.vector.tensor_tensor(out=gmax[:], in0=tmx[:, :H],
                                        in1=tmx[:, H:], op=mybir.AluOpType.max)
                nc.vector.tensor_tensor(out=tsm[:], in0=ht[:, :half],
                                        in1=ht[:, half:], op=mybir.AluOpType.add)
                nc.vector.tensor_tensor(out=gsum[:], in0=tsm[:, :H],
                                        in1=tsm[:, H:], op=mybir.AluOpType.add)
                # segment means accumulate on PE (weights already carry 1/cnt)
                for nh in range(2):
                    nc.tensor.matmul(
                        mean_ps[:, nh * 512:(nh + 1) * 512],
                        lhsT=mem_sb[:, t, :],
                        rhs=gsum[:, nh * 512:(nh + 1) * 512],
                        start=(t == 0),
                        stop=(t == ntiles - 1),
                    )
                # transpose the max partials: [group, h] -> [h, group]
                trp = trpp.tile([128, H], F32, tag="trp")
                for hb in range(HB):
                    nc.tensor.transpose(
                        trp[:, hb * 128:(hb + 1) * 128],
                        gmax[:, hb * 128:(hb + 1) * 128],
                        ident[:],
                    )
                nc.scalar.copy(
                    out=trmax[:, :, t * 128:(t + 1) * 128],
                    in_=trp[:].rearrange("p (b g) -> p b g", g=128),
                )

            # means: PSUM -> SBUF, then transpose to [h, slot] layout
            means = smallp.tile([seg_cap, D_OUT], F32)
            nc.scalar.copy(out=means[:], in_=mean_ps[:])
            tr2 = trpp.tile([128, HB * seg_cap], F32, tag="tr2")
            for hb in range(HB):
                nc.tensor.transpose(
                    tr2[:, hb * seg_cap:(hb + 1) * seg_cap],
                    means[:, hb * 128:(hb + 1) * 128],
                    ident[:seg_cap, :seg_cap],
                )
            meansT = smallp.tile([128, HB, seg_cap], F32)
            nc.scalar.copy(
                out=meansT[:],
                in_=tr2[:].rearrange("p (b j) -> p b j", j=seg_cap),
            )

            # per-segment max over its group range (all h-blocks at once)
            maxT = smallp.tile([128, seg_cap, HB], F32)
            for j in range(seg_cap):
                a, l = int(A[j]), int(L[j])
                nc.vector.reduce_max(
                    out=maxT[:, j, :],
                    in_=trmax[:, :, a:a + l],
                    axis=mybir.AxisListType.X,
                )

            # GEMM: out[slot, n] = sum_k featT[k, slot] * wt[k, n]
            out_ps = accp.tile([seg_cap, D_OUT], F32, tag="acc")
            for nh in range(2):
                nsl = slice(nh * 512, (nh + 1) * 512)
                for kb in range(2 * HB):
                    lhsT = maxT[:, :, kb] if kb < HB else meansT[:, kb - HB, :]
                    nc.tensor.matmul(
                        out_ps[:, nsl],
                        lhsT=lhsT,
                        rhs=wt_sb[:, kb, nsl],
                        start=(kb == 0),
                        stop=(kb == 2 * HB - 1),
                    )

            osb = smallp.tile([seg_cap, D_OUT], F32)
            nc.vector.tensor_add(out=osb[:], in0=out_ps[:], in1=brep_sb[:])
            osb2 = smallp.tile([seg_cap, D_OUT], F32)
            nc.scalar.activation(osb2[:], osb[:],
                                 mybir.ActivationFunctionType.Tanh)
            nc.sync.dma_start(out=out[:], in_=osb2[:])

    nc.compile()
    return nc


def _build_in_maps(sched, hidden_states, W, b):
    seg_cap, ntiles = sched["seg_cap"], sched["ntiles"]
    flat = np.ascontiguousarray(
        np.asarray(hidden_states, dtype=np.float32)).reshape(B * S, H)
    wt_np = np.ascontiguousarray(np.asarray(W, dtype=np.float32).T)  # [2H, D]
    brep_np = np.ascontiguousarray(
        np.broadcast_to(np.asarray(b, dtype=np.float32), (seg_cap, D_OUT)))

    in_maps = []
    for c in range(NCORES):
        idx = sched["tok_idx"][c]
        stream = np.zeros((sched["ntok"], H), dtype=np.float32)
        valid = idx >= 0
        stream[valid] = flat[idx[valid]]
        memc = np.ascontiguousarray(
            sched["member"][c].reshape(128, ntiles, seg_cap))
        in_maps.append({
            "hid": stream,
            "mem": memc,
            "wt": wt_np,
            "brep": brep_np,
        })
    return in_maps


def kernel(hidden_states, W, b, turns, parts):
    parts = np.asarray(parts)
    turns = np.asarray(turns)

    sched = _build_schedule(parts, turns)
    nc = _build_program(sched["ntiles"], sched["seg_cap"],
                        sched["A"], sched["L"])
    in_maps = _build_in_maps(sched, hidden_states, W, b)

    res = run_bass_kernel_spmd(nc, in_maps, list(range(NCORES)))

    full = np.zeros((sched["nrows"], D_OUT), dtype=np.float32)
    for c in range(NCORES):
        oc = res.results[c]["out"]
        for j in range(sched["seg_cap"]):
            g = sched["out_map"][c, j]
            if g >= 0:
                full[g] = oc[j]
    return full
